# revision 26
# baseline (speedup 1.0000x reference)
"""PixelFlow (bilinear warp + visibility gating) Trainium2 Bass kernel.

Full inputs: img [16,512,512,3] f32, flows [16,512,512,3] f32.
Sharding: pure data parallel, 2 images per NeuronCore across 8 cores.

Per-core algorithm (B=2 images):
  1. For each image, prepack a patch table P in DRAM: P[y*512+x] holds the
     12 floats [ch=3][corner=4] = (img[y,x], img[y,x+1], img[y+1,x],
     img[y+1,x+1]) per channel, corner innermost.  48B contiguous per pixel
     -> the whole bilinear stencil is ONE indirect-DMA descriptor.
  2. Per 128-row tile: compute source coords px,py bit-exactly like the
     reference, floor (f32->i32 trunc + negative fixup), clip, bilinear
     weights with collision-zero factors (x1c-x0c)/(y1c-y0c) in {0,1} which
     reproduce the TF bilinear_sampler out-of-range/edge semantics even
     though the gathered edge junk differs from the reference's clamped
     re-reads.  tanh visibility is folded into the y-weights.
  3. indirect_dma_start gathers 512*128 patches; one fused multiply by the
     per-pixel 4-corner weight vector (channel-broadcast via a 0-stride AP)
     and a tensor_reduce over the innermost corner axis produce the output.
"""

import numpy as np

import concourse.bass as bass
import concourse.tile as tile
from concourse import mybir
from concourse.bass import IndirectOffsetOnAxis

F32 = mybir.dt.float32
I32 = mybir.dt.int32
AL = mybir.AluOpType
ACTF = mybir.ActivationFunctionType
AX = mybir.AxisListType

NCORES = 8
BPC = 2          # images per core
H = W = 512
C = 3
HW = H * W
P = 128          # partitions
NT = H // P      # row tiles per image


def build_program(split_waits: bool = True) -> bass.Bass:
    nc = bass.Bass()

    img = nc.dram_tensor("img", [BPC, H, W, C], F32, kind="ExternalInput")
    flows = nc.dram_tensor("flows", [BPC, H, W, C], F32, kind="ExternalInput")
    out = nc.dram_tensor("out", [BPC, H, W, C], F32, kind="ExternalOutput")
    # per-image patch tables (separate tensors: indirect-DMA in_ needs offset 0,
    # and separate tensors let image 1's build overlap image 0's gathers)
    ptabs = [nc.dram_tensor(f"ptab{b}", [HW, C * 4], F32) for b in range(BPC)]

    jrow_c = nc.inline_tensor(
        np.arange(W, dtype=np.float32).reshape(1, W), name="jrow"
    )
    icol_c = nc.inline_tensor(
        (np.arange(P, dtype=np.float32)[:, None] + P * np.arange(NT)[None, :])
        .astype(np.float32),
        name="icol",
    )

    with tile.TileContext(nc) as tc:
        with (
            tc.tile_pool(name="const", bufs=1) as cpool,
            tc.tile_pool(name="pband", bufs=1) as pband,
            tc.tile_pool(name="ptile", bufs=1) as ppool,
            tc.tile_pool(name="fio", bufs=2) as fio,
            tc.tile_pool(name="gath", bufs=2) as gpool,
            tc.tile_pool(name="wt", bufs=2) as wpool,
            tc.tile_pool(name="tmp", bufs=2) as tmp,
        ):
            jsb = cpool.tile([P, W], F32)
            jr = jrow_c[:]
            nc.gpsimd.dma_start(
                jsb[:], bass.AP(jr.tensor, jr.offset, [[0, P], jr.ap[1]])
            )
            isb = cpool.tile([P, NT], F32)
            nc.gpsimd.dma_start(isb[:], icol_c[:])
            jbc = jsb[:]

            def T(name, dt=F32):
                return tmp.tile([P, W], dt, tag=name, name=name)

            cmap = {}

            def constap(v):
                v = float(v)
                if v not in cmap:
                    ct = cpool.tile(
                        [P, 1], F32, name=f"cst{len(cmap)}", tag=f"cst{v}"
                    )
                    nc.vector.memset(ct[:], v)
                    cmap[v] = ct[:]
                return cmap[v]

            def build_p(b, t):
                r0 = t * P
                band = pband.tile([P, W + 1, C], F32, tag="band")
                nc.vector.memset(band[:, W : W + 1, :], 0.0)
                nc.gpsimd.dma_start(band[:, 0:W, :], img[b, r0 : r0 + P, :, :])
                bsh = pband.tile([P, W + 1, C], F32, tag="bsh")
                if t < NT - 1:
                    nc.vector.memset(bsh[:, W : W + 1, :], 0.0)
                    nc.gpsimd.dma_start(
                        bsh[:, 0:W, :], img[b, r0 + 1 : r0 + P + 1, :, :]
                    )
                else:
                    nc.vector.memset(bsh[:], 0.0)
                    nc.gpsimd.dma_start(
                        bsh[0 : P - 1, 0:W, :], img[b, r0 + 1 : H, :, :]
                    )

                pt = ppool.tile([P, W, C, 4], F32, tag="pt")
                nc.vector.tensor_copy(pt[:, :, :, 0], band[:, 0:W, :])
                nc.vector.tensor_copy(pt[:, :, :, 1], band[:, 1 : W + 1, :])
                nc.vector.tensor_copy(pt[:, :, :, 2], bsh[:, 0:W, :])
                nc.vector.tensor_copy(pt[:, :, :, 3], bsh[:, 1 : W + 1, :])
                dst = ptabs[b][t * P * W : (t + 1) * P * W, :]
                nc.gpsimd.dma_start(dst, pt[:])

            def coord_chain(p32, eng_scalar):
                """floor+clip+weights for one axis given p32 = px or py tile.
                Returns (c0, w0, w1): clipped low coord, low/high weights
                (with the collision-zero factor folded in)."""
                ci = T(f"ci_{eng_scalar}", I32)
                nc.vector.tensor_copy(ci[:], p32[:])
                c0 = T(f"c0_{eng_scalar}")
                nc.vector.tensor_copy(c0[:], ci[:])
                nm = T(f"nm_{eng_scalar}")
                nc.vector.tensor_tensor(nm[:], c0[:], p32[:], AL.is_gt)
                nc.vector.tensor_tensor(c0[:], c0[:], nm[:], AL.subtract)
                c1 = T(f"c1_{eng_scalar}")
                nc.vector.tensor_scalar(c1[:], c0[:], 1.0, 0.0, AL.add, AL.max)
                nc.vector.tensor_scalar(
                    c1[:], c1[:], float(W - 1), scalar2=None, op0=AL.min
                )
                nc.vector.tensor_scalar(
                    c0[:], c0[:], 0.0, float(W - 1), AL.max, AL.min
                )
                d01 = T(f"d01_{eng_scalar}")
                nc.vector.tensor_tensor(d01[:], c1[:], c0[:], AL.subtract)
                whi = T(f"whi_{eng_scalar}")
                nc.vector.tensor_tensor(whi[:], p32[:], c0[:], AL.subtract)
                nc.vector.tensor_tensor(whi[:], whi[:], d01[:], AL.mult)
                wlo = T(f"wlo_{eng_scalar}")
                nc.vector.tensor_tensor(wlo[:], d01[:], whi[:], AL.subtract)
                return c0, wlo, whi

            def warp_tile(b, t):
                r0 = t * P
                ft = fio.tile([P, W, C], F32, tag="F")
                nc.gpsimd.dma_start(ft[:], flows[b, r0 : r0 + P, :, :])
                xf = ft[:, :, 0]
                yf = ft[:, :, 1]
                vm = ft[:, :, 2]

                # px chain (bit-exact vs reference) on DVE
                px = T("px")
                nc.vector.tensor_tensor(px[:], xf, jbc, AL.add)
                nc.vector.tensor_scalar(
                    px[:], px[:], 1.0 / W, -0.5, AL.mult, AL.add
                )
                nc.vector.tensor_scalar(px[:], px[:], 2.0, 1.0, AL.mult, AL.add)
                nc.vector.tensor_scalar(
                    px[:], px[:], (W - 1) / 2.0, scalar2=None, op0=AL.mult
                )
                # py chain on ACT (per-partition row index as bias)
                py = T("py")
                nc.scalar.activation(
                    py[:], yf, ACTF.Identity, bias=isb[:, t : t + 1], scale=1.0
                )
                nc.scalar.activation(
                    py[:], py[:], ACTF.Identity,
                    bias=constap(-0.5), scale=constap(1.0 / H),
                )
                nc.scalar.activation(
                    py[:], py[:], ACTF.Identity,
                    bias=1.0, scale=constap(2.0),
                )
                nc.scalar.activation(
                    py[:], py[:], ACTF.Identity,
                    bias=0.0, scale=constap((H - 1) / 2.0),
                )

                x0, wxl, wxh = coord_chain(px, "x")
                y0, wyl, wyh = coord_chain(py, "y")

                # vis = tanh(v)+1 folded into the y weights
                vh = T("vh")
                nc.scalar.activation(vh[:], vm, ACTF.Tanh)
                nc.vector.scalar_tensor_tensor(
                    wyl[:], vh[:], 1.0, wyl[:], AL.add, AL.mult
                )
                nc.vector.scalar_tensor_tensor(
                    wyh[:], vh[:], 1.0, wyh[:], AL.add, AL.mult
                )

                wt_t = wpool.tile([P, W, 4], F32, tag="W")
                nc.vector.tensor_tensor(wt_t[:, :, 0], wxl[:], wyl[:], AL.mult)
                nc.vector.tensor_tensor(wt_t[:, :, 1], wxh[:], wyl[:], AL.mult)
                nc.vector.tensor_tensor(wt_t[:, :, 2], wxl[:], wyh[:], AL.mult)
                nc.vector.tensor_tensor(wt_t[:, :, 3], wxh[:], wyh[:], AL.mult)

                idf = T("idf")
                nc.vector.scalar_tensor_tensor(
                    idf[:], y0[:], float(W), x0[:], AL.mult, AL.add
                )
                idx = T("idx", I32)
                nc.vector.tensor_copy(idx[:], idf[:])

                g = gpool.tile([P, W * C * 4], F32, tag="G")
                # HW indirect DMA honors ONE index per partition: 512
                # instructions per tile, each gathering 128 patches.
                for s in range(W):
                    nc.gpsimd.indirect_dma_start(
                        out=g[:, s * 12 : (s + 1) * 12],
                        out_offset=None,
                        in_=ptabs[b][:],
                        in_offset=IndirectOffsetOnAxis(
                            ap=idx[:, s : s + 1], axis=0
                        ),
                    )
                g4 = g[:].rearrange("p (x c k) -> p x c k", x=W, c=C, k=4)

                # weight AP broadcast over channels: [P][W][0 x C][4]
                wap = wt_t[:]
                wbc = bass.AP(
                    wap.tensor,
                    wap.offset,
                    [wap.ap[0], wap.ap[1], [0, C], wap.ap[2]],
                )
                nc.vector.tensor_tensor(g4, g4, wbc, AL.mult)
                o = fio.tile([P, W, C], F32, tag="O")
                nc.vector.tensor_reduce(o[:], g4, axis=AX.X, op=AL.add)
                nc.gpsimd.dma_start(out[b, r0 : r0 + P, :, :], o[:])

            for b in range(BPC):
                for t in range(NT):
                    build_p(b, t)
                for t in range(NT):
                    warp_tile(b, t)

    if split_waits:
        _split_excess_waits(nc)
    return nc


def _split_excess_waits(nc, max_waits=1):
    """walrus codegen accepts at most 2 sync-wait commands per instruction.
    Move excess waits onto InstNoOps inserted just before the instruction in
    the same engine stream (in-order issue makes this equivalent)."""
    k = 0
    for bb in nc.main_func.blocks:
        new_insts = []
        for ins in bb.instructions:
            si = ins.sync_info
            waits = list(si.on_wait) if si is not None and si.on_wait else []
            cap = max_waits
            if len(waits) > cap:
                keep = waits[-cap:]
                excess = waits[:-cap]
                for i in range(0, len(excess), max_waits):
                    nop = mybir.InstNoOp(
                        name=f"wsplit-{k}",
                        engine=ins.engine,
                        sync_info=mybir.SyncInfo(
                            on_wait=excess[i : i + max_waits], on_update=[]
                        ),
                    )
                    k += 1
                    new_insts.append(nop)
                ins.sync_info = mybir.SyncInfo(
                    on_wait=keep, on_update=list(si.on_update or [])
                )
            new_insts.append(ins)
        bb.instructions = new_insts
    return nc


_PROGRAM = None


def _program():
    global _PROGRAM
    if _PROGRAM is None:
        _PROGRAM = build_program()
    return _PROGRAM


def kernel(img: np.ndarray, flows: np.ndarray) -> np.ndarray:
    from concourse.bass_utils import run_bass_kernel_spmd

    img = np.ascontiguousarray(np.asarray(img, dtype=np.float32))
    flows = np.ascontiguousarray(np.asarray(flows, dtype=np.float32))
    assert img.shape == (NCORES * BPC, H, W, C), img.shape

    nc = _program()
    in_maps = [
        {
            "img": img[k * BPC : (k + 1) * BPC],
            "flows": flows[k * BPC : (k + 1) * BPC],
        }
        for k in range(NCORES)
    ]
    res = run_bass_kernel_spmd(nc, in_maps, list(range(NCORES)))
    return np.concatenate([r["out"] for r in res.results], axis=0)


# revision 29
# speedup vs baseline: 710.9812x; 710.9812x over previous
"""PixelFlow (bilinear warp + visibility gating) Trainium2 Bass kernel.

Full inputs: img [16,512,512,3] f32, flows [16,512,512,3] f32.
Sharding: pure data parallel, 2 images per NeuronCore across 8 cores.

Per-core algorithm (B=2 images):
  1. For each image, prepack a patch table P in DRAM: P[y*512+x] holds the
     12 floats [ch=3][corner=4] = (img[y,x], img[y,x+1], img[y+1,x],
     img[y+1,x+1]) per channel, corner innermost.  48B contiguous per pixel
     -> the whole bilinear stencil is ONE indirect-DMA descriptor.
  2. Per 128-row tile: compute source coords px,py bit-exactly like the
     reference, floor (f32->i32 trunc + negative fixup), clip, bilinear
     weights with collision-zero factors (x1c-x0c)/(y1c-y0c) in {0,1} which
     reproduce the TF bilinear_sampler out-of-range/edge semantics even
     though the gathered edge junk differs from the reference's clamped
     re-reads.  tanh visibility is folded into the y-weights.
  3. indirect_dma_start gathers 512*128 patches; one fused multiply by the
     per-pixel 4-corner weight vector (channel-broadcast via a 0-stride AP)
     and a tensor_reduce over the innermost corner axis produce the output.
"""

import numpy as np

import concourse.bass as bass
import concourse.tile as tile
from concourse import mybir
from concourse.bass import IndirectOffsetOnAxis

F32 = mybir.dt.float32
I32 = mybir.dt.int32
AL = mybir.AluOpType
ACTF = mybir.ActivationFunctionType
AX = mybir.AxisListType

NCORES = 8
BPC = 2          # images per core
H = W = 512
C = 3
HW = H * W
P = 128          # partitions
NT = H // P      # row tiles per image


def build_program(split_waits: bool = True) -> bass.Bass:
    nc = bass.Bass()

    img = nc.dram_tensor("img", [BPC, H, W, C], F32, kind="ExternalInput")
    flows = nc.dram_tensor("flows", [BPC, H, W, C], F32, kind="ExternalInput")
    out = nc.dram_tensor("out", [BPC, H, W, C], F32, kind="ExternalOutput")
    # per-image patch tables (separate tensors: indirect-DMA in_ needs offset 0,
    # and separate tensors let image 1's build overlap image 0's gathers)
    ptabs = [nc.dram_tensor(f"ptab{b}", [HW, C * 4], F32) for b in range(BPC)]

    jrow_c = nc.inline_tensor(
        np.arange(W, dtype=np.float32).reshape(1, W), name="jrow"
    )
    icol_c = nc.inline_tensor(
        (np.arange(P, dtype=np.float32)[:, None] + P * np.arange(NT)[None, :])
        .astype(np.float32),
        name="icol",
    )

    with tile.TileContext(nc) as tc:
        with (
            tc.tile_pool(name="const", bufs=1) as cpool,
            tc.tile_pool(name="pband", bufs=1) as pband,
            tc.tile_pool(name="ptile", bufs=1) as ppool,
            tc.tile_pool(name="fio", bufs=2) as fio,
            tc.tile_pool(name="gath", bufs=2) as gpool,
            tc.tile_pool(name="wt", bufs=2) as wpool,
            tc.tile_pool(name="tmp", bufs=2) as tmp,
        ):
            jsb = cpool.tile([P, W], F32)
            jr = jrow_c[:]
            nc.sync.dma_start(
                jsb[:], bass.AP(jr.tensor, jr.offset, [[0, P], jr.ap[1]])
            )
            isb = cpool.tile([P, NT], F32)
            nc.sync.dma_start(isb[:], icol_c[:])
            jbc = jsb[:]

            def T(name, dt=F32):
                return tmp.tile([P, W], dt, tag=name, name=name)

            cmap = {}

            def constap(v):
                v = float(v)
                if v not in cmap:
                    ct = cpool.tile(
                        [P, 1], F32, name=f"cst{len(cmap)}", tag=f"cst{v}"
                    )
                    nc.vector.memset(ct[:], v)
                    cmap[v] = ct[:]
                return cmap[v]

            def build_p(b, t):
                r0 = t * P
                band = pband.tile([P, W + 1, C], F32, tag="band")
                nc.vector.memset(band[:, W : W + 1, :], 0.0)
                nc.sync.dma_start(band[:, 0:W, :], img[b, r0 : r0 + P, :, :])
                bsh = pband.tile([P, W + 1, C], F32, tag="bsh")
                if t < NT - 1:
                    nc.vector.memset(bsh[:, W : W + 1, :], 0.0)
                    nc.sync.dma_start(
                        bsh[:, 0:W, :], img[b, r0 + 1 : r0 + P + 1, :, :]
                    )
                else:
                    nc.vector.memset(bsh[:], 0.0)
                    nc.sync.dma_start(
                        bsh[0 : P - 1, 0:W, :], img[b, r0 + 1 : H, :, :]
                    )

                pt = ppool.tile([P, W, C, 4], F32, tag="pt")
                nc.vector.tensor_copy(pt[:, :, :, 0], band[:, 0:W, :])
                nc.vector.tensor_copy(pt[:, :, :, 1], band[:, 1 : W + 1, :])
                nc.vector.tensor_copy(pt[:, :, :, 2], bsh[:, 0:W, :])
                nc.vector.tensor_copy(pt[:, :, :, 3], bsh[:, 1 : W + 1, :])
                dst = ptabs[b][t * P * W : (t + 1) * P * W, :]
                nc.sync.dma_start(dst, pt[:])

            def coord_chain(p32, eng_scalar):
                """floor+clip+weights for one axis given p32 = px or py tile.
                Returns (c0, w0, w1): clipped low coord, low/high weights
                (with the collision-zero factor folded in)."""
                ci = T(f"ci_{eng_scalar}", I32)
                nc.vector.tensor_copy(ci[:], p32[:])
                c0 = T(f"c0_{eng_scalar}")
                nc.vector.tensor_copy(c0[:], ci[:])
                nm = T(f"nm_{eng_scalar}")
                nc.vector.tensor_tensor(nm[:], c0[:], p32[:], AL.is_gt)
                nc.vector.tensor_tensor(c0[:], c0[:], nm[:], AL.subtract)
                c1 = T(f"c1_{eng_scalar}")
                nc.vector.tensor_scalar(c1[:], c0[:], 1.0, 0.0, AL.add, AL.max)
                nc.vector.tensor_scalar(
                    c1[:], c1[:], float(W - 1), scalar2=None, op0=AL.min
                )
                nc.vector.tensor_scalar(
                    c0[:], c0[:], 0.0, float(W - 1), AL.max, AL.min
                )
                d01 = T(f"d01_{eng_scalar}")
                nc.vector.tensor_tensor(d01[:], c1[:], c0[:], AL.subtract)
                whi = T(f"whi_{eng_scalar}")
                nc.vector.tensor_tensor(whi[:], p32[:], c0[:], AL.subtract)
                nc.vector.tensor_tensor(whi[:], whi[:], d01[:], AL.mult)
                wlo = T(f"wlo_{eng_scalar}")
                nc.vector.tensor_tensor(wlo[:], d01[:], whi[:], AL.subtract)
                return c0, wlo, whi

            def warp_tile(b, t):
                r0 = t * P
                ft = fio.tile([P, W, C], F32, tag="F")
                nc.sync.dma_start(ft[:], flows[b, r0 : r0 + P, :, :])
                xf = ft[:, :, 0]
                yf = ft[:, :, 1]
                vm = ft[:, :, 2]

                # px chain (bit-exact vs reference) on DVE
                px = T("px")
                nc.vector.tensor_tensor(px[:], xf, jbc, AL.add)
                nc.vector.tensor_scalar(
                    px[:], px[:], 1.0 / W, -0.5, AL.mult, AL.add
                )
                nc.vector.tensor_scalar(px[:], px[:], 2.0, 1.0, AL.mult, AL.add)
                nc.vector.tensor_scalar(
                    px[:], px[:], (W - 1) / 2.0, scalar2=None, op0=AL.mult
                )
                # py chain on ACT (per-partition row index as bias)
                py = T("py")
                nc.scalar.activation(
                    py[:], yf, ACTF.Identity, bias=isb[:, t : t + 1], scale=1.0
                )
                nc.scalar.activation(
                    py[:], py[:], ACTF.Identity,
                    bias=constap(-0.5), scale=constap(1.0 / H),
                )
                nc.scalar.activation(
                    py[:], py[:], ACTF.Identity,
                    bias=1.0, scale=constap(2.0),
                )
                nc.scalar.activation(
                    py[:], py[:], ACTF.Identity,
                    bias=0.0, scale=constap((H - 1) / 2.0),
                )

                x0, wxl, wxh = coord_chain(px, "x")
                y0, wyl, wyh = coord_chain(py, "y")

                # vis = tanh(v)+1 folded into the y weights
                vh = T("vh")
                nc.scalar.activation(vh[:], vm, ACTF.Tanh)
                nc.vector.scalar_tensor_tensor(
                    wyl[:], vh[:], 1.0, wyl[:], AL.add, AL.mult
                )
                nc.vector.scalar_tensor_tensor(
                    wyh[:], vh[:], 1.0, wyh[:], AL.add, AL.mult
                )

                wt_t = wpool.tile([P, W, 4], F32, tag="W")
                nc.vector.tensor_tensor(wt_t[:, :, 0], wxl[:], wyl[:], AL.mult)
                nc.vector.tensor_tensor(wt_t[:, :, 1], wxh[:], wyl[:], AL.mult)
                nc.vector.tensor_tensor(wt_t[:, :, 2], wxl[:], wyh[:], AL.mult)
                nc.vector.tensor_tensor(wt_t[:, :, 3], wxh[:], wyh[:], AL.mult)

                idf = T("idf")
                nc.vector.scalar_tensor_tensor(
                    idf[:], y0[:], float(W), x0[:], AL.mult, AL.add
                )
                idx = T("idx", I32)
                nc.vector.tensor_copy(idx[:], idf[:])

                g = gpool.tile([P, W * C * 4], F32, tag="G")
                # HW indirect DMA honors ONE index per partition: 512
                # instructions per tile, each gathering 128 patches.
                for s in range(W):
                    nc.gpsimd.indirect_dma_start(
                        out=g[:, s * 12 : (s + 1) * 12],
                        out_offset=None,
                        in_=ptabs[b][:],
                        in_offset=IndirectOffsetOnAxis(
                            ap=idx[:, s : s + 1], axis=0
                        ),
                    )
                g4 = g[:].rearrange("p (x c k) -> p x c k", x=W, c=C, k=4)

                # weight AP broadcast over channels: [P][W][0 x C][4]
                wap = wt_t[:]
                wbc = bass.AP(
                    wap.tensor,
                    wap.offset,
                    [wap.ap[0], wap.ap[1], [0, C], wap.ap[2]],
                )
                nc.vector.tensor_tensor(g4, g4, wbc, AL.mult)
                o = fio.tile([P, W, C], F32, tag="O")
                nc.vector.tensor_reduce(o[:], g4, axis=AX.X, op=AL.add)
                nc.sync.dma_start(out[b, r0 : r0 + P, :, :], o[:])

            for b in range(BPC):
                for t in range(NT):
                    build_p(b, t)
                for t in range(NT):
                    warp_tile(b, t)

    if split_waits:
        _split_excess_waits(nc)
    return nc


def _split_excess_waits(nc, max_waits=1):
    """walrus codegen accepts at most 2 sync-wait commands per instruction.
    Move excess waits onto InstNoOps inserted just before the instruction in
    the same engine stream (in-order issue makes this equivalent)."""
    k = 0
    for bb in nc.main_func.blocks:
        new_insts = []
        for ins in bb.instructions:
            si = ins.sync_info
            waits = list(si.on_wait) if si is not None and si.on_wait else []
            cap = max_waits
            if len(waits) > cap:
                keep = waits[-cap:]
                excess = waits[:-cap]
                for i in range(0, len(excess), max_waits):
                    nop = mybir.InstNoOp(
                        name=f"wsplit-{k}",
                        engine=ins.engine,
                        sync_info=mybir.SyncInfo(
                            on_wait=excess[i : i + max_waits], on_update=[]
                        ),
                    )
                    k += 1
                    new_insts.append(nop)
                ins.sync_info = mybir.SyncInfo(
                    on_wait=keep, on_update=list(si.on_update or [])
                )
            new_insts.append(ins)
        bb.instructions = new_insts
    return nc


_PROGRAM = None


def _program():
    global _PROGRAM
    if _PROGRAM is None:
        _PROGRAM = build_program()
    return _PROGRAM


def kernel(img: np.ndarray, flows: np.ndarray) -> np.ndarray:
    from concourse.bass_utils import run_bass_kernel_spmd

    img = np.ascontiguousarray(np.asarray(img, dtype=np.float32))
    flows = np.ascontiguousarray(np.asarray(flows, dtype=np.float32))
    assert img.shape == (NCORES * BPC, H, W, C), img.shape

    nc = _program()
    in_maps = [
        {
            "img": img[k * BPC : (k + 1) * BPC],
            "flows": flows[k * BPC : (k + 1) * BPC],
        }
        for k in range(NCORES)
    ]
    res = run_bass_kernel_spmd(nc, in_maps, list(range(NCORES)))
    return np.concatenate([r["out"] for r in res.results], axis=0)
